# revision 2
# baseline (speedup 1.0000x reference)
"""Self-contained Trainium2 kernel for nn_Attention_42984032699151.

Dense GQA attention (B=1, T=2048, DIM=4096, 32 q heads, 8 kv heads,
head_dim=128, RoPE, causal) tensor-parallel over 8 NeuronCores: core i owns
kv head i and q heads 4i..4i+3 (wq/wk/wv column-sharded, wo row-sharded);
the wo all-reduce is done on host by summing the 8 partial outputs.

Per-core kernel (all matmuls in float32r: full PE rate at free-dim>=256,
~1.4e-4 rel err):
  1. qkvT[m] = (wqkv m-tile).T @ x           via xT tiles, PSUM k-accum
  2. RoPE applied in transposed layout: rot(q) = q*C + (Pswap@q)*S2,
     Pswap a 128x128 pair-swap permutation on the PE; v PE-transposed to
     natural layout.
  3. Flash-style causal attention, scores computed directly transposed
     (S^T = kT.T-block @ qT), additive -3e4 masks on diagonal blocks, exp on
     ScalarE without max-subtraction (scores bounded ~|12|), attn@V and row
     sums (ones-column matmul) accumulated in PSUM, late 1/d normalization
     broadcast via a K=1 matmul.
  4. outT partial = wo-block.T @ ctxT, streamed to DRAM.
"""

import numpy as np
import orjson

import concourse.bass as bass
import concourse.tile as tile
from concourse import mybir
from concourse.bass_utils import run_bass_kernel_spmd

F32 = mybir.dt.float32
F32R = mybir.dt.float32r
EXP = mybir.ActivationFunctionType.Exp

T, DIM = 2048, 4096
HD = 128          # head dim
NQ = 4            # q heads per core
NM = 6            # phase-1 m-tiles: 4 q + 1 k + 1 v
QKV = (NQ + 2) * HD
SC = 512          # score/ctx tq-chunk width
TPB = SC // 128
N_CORES = 8

_MAX_WAITS = 1


def _split_waits_in_bir(bir_bytes: bytes) -> bytes:
    """walrus rejects >1 sem-wait per instruction ("Too many sync wait
    commands"); hoist excess waits onto Drain instructions inserted before
    the offender (sequential waiting is equivalent)."""
    d = orjson.loads(bir_bytes)
    changed = False
    for fn in d.get("functions", []):
        for blk in fn.get("blocks", []):
            insts = blk.get("instructions") or []
            new_insts = []
            for inst in insts:
                si = inst.get("sync_info") or {}
                waits = si.get("on_wait") or []
                if len(waits) > _MAX_WAITS:
                    changed = True
                    extra = waits[: len(waits) - _MAX_WAITS]
                    keep = waits[len(waits) - _MAX_WAITS:]
                    for j in range(0, len(extra), _MAX_WAITS):
                        chunk = extra[j : j + _MAX_WAITS]
                        new_insts.append({
                            "name": f"{inst['name']}.w{j}",
                            "opcode": "Drain",
                            "engine": inst["engine"],
                            "ins": [],
                            "outs": [],
                            "is_reset_sema": False,
                            "debug": inst.get("debug", 0),
                            "sync_info": {"on_update": [], "on_wait": chunk},
                        })
                    si["on_wait"] = keep
                    inst["sync_info"] = si
                new_insts.append(inst)
            blk["instructions"] = new_insts
    return orjson.dumps(d) if changed else bir_bytes


_installed = False


def _install_fixups():
    global _installed
    if _installed:
        return
    _installed = True

    import concourse.bass2jax as b2j
    from concourse.bass_utils import compile_bir_kernel as _orig

    def wrapped(ant_bir_str, compile_dir_path, neff_name="kernel.neff", **kw):
        ant_bir_str = _split_waits_in_bir(ant_bir_str)
        return _orig(ant_bir_str, compile_dir_path, neff_name=neff_name, **kw)

    b2j.compile_bir_kernel = wrapped

    # Recreate the NTFF profile hook module if the image lacks it (harmless
    # if profiling is never requested).
    try:
        import sys
        import types

        import antenv

        if "antenv.axon_hooks" not in sys.modules:
            mod = types.ModuleType("antenv.axon_hooks")
            mod._hook = None
            mod.set_axon_ntff_profile_hook = lambda h: setattr(mod, "_hook", h)
            mod.get_axon_ntff_profile_hook = lambda: mod._hook
            sys.modules["antenv.axon_hooks"] = mod
            antenv.axon_hooks = mod
        from antenv.axon_hooks import (
            get_axon_ntff_profile_hook,
            set_axon_ntff_profile_hook,
        )

        if get_axon_ntff_profile_hook() is None:
            from trn_agent_boot.trn_boot import _ntff_profile_via_ctypes

            set_axon_ntff_profile_hook(
                _ntff_profile_via_ctypes("/opt/axon/libaxon_pjrt.so"))
    except Exception:
        pass


def build(T=T, DIM=DIM, xt_lookahead=4):
    KT = DIM // 128
    KH = KT // 2
    NSC = T // SC
    NTK = T // 128

    nc = bass.Bass()
    xT = nc.dram_tensor("xT", [DIM, T], F32R, kind="ExternalInput")
    wqkv = nc.dram_tensor("wqkv", [DIM, QKV], F32R, kind="ExternalInput")
    wo = nc.dram_tensor("wo", [NQ * HD, DIM], F32R, kind="ExternalInput")
    ropeC = nc.dram_tensor("ropeC", [128, T], F32, kind="ExternalInput")
    ropeS2 = nc.dram_tensor("ropeS2", [128, T], F32, kind="ExternalInput")
    masks = nc.dram_tensor("masks", [128, TPB, SC], F32, kind="ExternalInput")
    pswap = nc.dram_tensor("pswap", [128, 128], F32R, kind="ExternalInput")
    ident = nc.dram_tensor("ident", [128, 128], F32R, kind="ExternalInput")
    outT = nc.dram_tensor("outT", [DIM, T], F32, kind="ExternalOutput")

    with tile.TileContext(nc) as tc:
        with tc.tile_pool(name="persist", bufs=1) as pp:
            qkvT = [pp.tile([128, T], F32R, tag=f"qkvT{m}", name=f"qkvT{m}")
                    for m in range(NM)]
            v_nat = pp.tile([128, NTK, 128], F32R, tag="v_nat")
            ones_col = pp.tile([128, 1], F32R, tag="ones_col")
            ones_row = pp.tile([1, 128], F32R, tag="ones_row")
            ones_f32 = pp.tile([128, 1], F32, tag="ones_f32")
            ones_row_f32 = pp.tile([1, 128], F32, tag="ones_row_f32")
            nc.vector.memset(ones_f32, 1.0)
            nc.vector.memset(ones_row_f32, 1.0)
            nc.vector.tensor_copy(out=ones_col[:], in_=ones_f32[:])
            nc.vector.tensor_copy(out=ones_row[:], in_=ones_row_f32[:])

            # ---- Phase 1: QKV projection into transposed layout ----
            with tc.tile_pool(name="wq", bufs=1) as wqp, \
                 tc.tile_pool(name="xt", bufs=KH + xt_lookahead) as xtp, \
                 tc.tile_pool(name="ps1", bufs=1, space="PSUM") as ps1:
                wq_sb = wqp.tile([128, KT, QKV], F32R, tag="wqkv")
                nc.sync.dma_start(
                    out=wq_sb, in_=wqkv.ap().rearrange("(k p) n -> p k n", p=128))
                for c in range(T // SC):
                    csl = slice(c * SC, (c + 1) * SC)
                    pss = [ps1.tile([128, SC], F32, tag=f"pm{m}", name=f"pm{m}_{c}")
                           for m in range(NM)]
                    for kh in range(2):
                        xts = []
                        for k in range(KH):
                            kk = kh * KH + k
                            xt_t = xtp.tile([128, SC], F32R, tag="xt")
                            nc.sync.dma_start(
                                out=xt_t, in_=xT.ap()[kk * 128:(kk + 1) * 128, csl])
                            xts.append(xt_t)
                        for k in range(KH):       # k-outer: frees xt slots early
                            kk = kh * KH + k
                            for m in range(NM):
                                nc.tensor.matmul(
                                    pss[m][:],
                                    wq_sb[:, kk, m * 128:(m + 1) * 128],
                                    xts[k][:],
                                    start=(kk == 0), stop=(kk == KT - 1))
                    for m in range(NM):
                        nc.vector.tensor_copy(out=qkvT[m][:, csl], in_=pss[m][:])

            # ---- Phases 2-4 (phase-1 pools freed) ----
            with tc.tile_pool(name="persist2", bufs=1) as pp2:
                ctxT = [pp2.tile([128, T], F32R, tag=f"ctxT{h}", name=f"ctxT{h}")
                        for h in range(NQ)]
                with tc.tile_pool(name="tbl", bufs=1) as tbl, \
                     tc.tile_pool(name="tmp2", bufs=3) as tmp2, \
                     tc.tile_pool(name="ps2", bufs=2, space="PSUM") as ps2:
                    C_sb = tbl.tile([128, T], F32, tag="C")
                    S2_sb = tbl.tile([128, T], F32, tag="S2")
                    psw_sb = tbl.tile([128, 128], F32R, tag="psw")
                    id_sb = tbl.tile([128, 128], F32R, tag="id")
                    nc.sync.dma_start(out=C_sb, in_=ropeC.ap())
                    nc.sync.dma_start(out=S2_sb, in_=ropeS2.ap())
                    nc.sync.dma_start(out=psw_sb, in_=pswap.ap())
                    nc.sync.dma_start(out=id_sb, in_=ident.ap())
                    for m in range(NQ + 1):       # RoPE on q heads + k, in place
                        tgt = qkvT[m]
                        for c in range(T // SC):
                            csl = slice(c * SC, (c + 1) * SC)
                            ps_sw = ps2.tile([128, SC], F32, tag="sw")
                            nc.tensor.matmul(ps_sw[:], psw_sb[:], tgt[:, csl],
                                             start=True, stop=True)
                            tA = tmp2.tile([128, SC], F32, tag="tA")
                            nc.vector.tensor_mul(out=tA[:], in0=tgt[:, csl],
                                                 in1=C_sb[:, csl])
                            tB = tmp2.tile([128, SC], F32, tag="tB")
                            nc.vector.tensor_mul(out=tB[:], in0=ps_sw[:],
                                                 in1=S2_sb[:, csl])
                            nc.vector.tensor_add(out=tgt[:, csl], in0=tA[:],
                                                 in1=tB[:])
                    for t in range(NTK):          # v -> natural layout
                        ps_t = ps2.tile([128, 128], F32R, tag="tr")
                        nc.tensor.transpose(ps_t[:],
                                            qkvT[5][:, t * 128:(t + 1) * 128],
                                            id_sb[:])
                        nc.vector.tensor_copy(out=v_nat[:, t, :], in_=ps_t[:])

                # ---- Phase 3: causal attention ----
                with tc.tile_pool(name="msk", bufs=1) as mskp, \
                     tc.tile_pool(name="expp", bufs=4) as expp, \
                     tc.tile_pool(name="tmp3", bufs=3) as tmp3, \
                     tc.tile_pool(name="wo4", bufs=1) as wo4:
                  with tc.tile_pool(name="ps3", bufs=1, space="PSUM") as ps3:
                      msk_sb = mskp.tile([128, TPB, SC], F32, tag="masks")
                      nc.sync.dma_start(out=msk_sb, in_=masks.ap())
                      wo_sb = wo4.tile([128, NQ, DIM], F32R, tag="wo")
                      nc.sync.dma_start(
                          out=wo_sb,
                          in_=wo.ap().rearrange("(h p) d -> p h d", p=128))
                      kT = qkvT[4]
                      for h in range(NQ):
                          qh = qkvT[h]
                          for c in range(NSC):
                              csl = slice(c * SC, (c + 1) * SC)
                              ntk = (c + 1) * TPB
                              ps_ctx = ps3.tile([128, SC], F32, tag=f"ctx{c % 2}")
                              ps_d = ps3.tile([1, SC], F32, tag="d")
                              for tk in range(ntk):
                                  ps_s = ps3.tile([128, SC], F32, tag=f"S{tk % 2}")
                                  nc.tensor.matmul(
                                      ps_s[:], kT[:, tk * 128:(tk + 1) * 128],
                                      qh[:, csl], start=True, stop=True)
                                  off = tk - TPB * c
                                  if off >= 0:
                                      nc.vector.tensor_add(
                                          out=ps_s[:], in0=ps_s[:],
                                          in1=msk_sb[:, off, :])
                                  e = expp.tile([128, SC], F32R, tag="exp")
                                  nc.scalar.activation(out=e[:], in_=ps_s[:],
                                                       func=EXP)
                                  nc.tensor.matmul(
                                      ps_ctx[:], v_nat[:, tk, :], e[:],
                                      start=(tk == 0), stop=(tk == ntk - 1))
                                  nc.tensor.matmul(
                                      ps_d[:], ones_col[:], e[:],
                                      start=(tk == 0), stop=(tk == ntk - 1))
                              rec = tmp3.tile([1, SC], F32R, tag="rec")
                              with nc.allow_low_precision(
                                      reason="f32r is fp32-width"):
                                  nc.vector.reciprocal(out=rec[:], in_=ps_d[:])
                              ps_b = ps3.tile([128, SC], F32, tag="bc")
                              nc.tensor.matmul(ps_b[:], ones_row[:], rec[:],
                                               start=True, stop=True)
                              bc = tmp3.tile([128, SC], F32, tag="bc_sb")
                              nc.vector.tensor_copy(out=bc[:], in_=ps_b[:])
                              nc.vector.tensor_mul(out=ctxT[h][:, csl],
                                                   in0=ps_ctx[:], in1=bc[:])

                  # ---- Phase 4: output projection (partial) ----
                  with tc.tile_pool(name="outp", bufs=4) as outp, \
                       tc.tile_pool(name="ps4", bufs=1, space="PSUM") as ps4:
                      for dc in range(DIM // 128):
                          dsl = slice(dc * 128, (dc + 1) * 128)
                          ps_o = [ps4.tile([128, SC], F32, tag=f"o{c % 4}",
                                           name=f"o{dc}_{c}")
                                  for c in range(NSC)]
                          for h in range(NQ):
                              for c in range(NSC):
                                  nc.tensor.matmul(
                                      ps_o[c][:], wo_sb[:, h, dsl],
                                      ctxT[h][:, c * SC:(c + 1) * SC],
                                      start=(h == 0), stop=(h == NQ - 1))
                          for c in range(NSC):
                              ob = outp.tile([128, SC], F32, tag="ob")
                              nc.vector.tensor_copy(out=ob[:], in_=ps_o[c][:])
                              nc.sync.dma_start(
                                  out=outT.ap()[dsl, c * SC:(c + 1) * SC],
                                  in_=ob[:])
    return nc


def host_prep(x, rope_cos, rope_sin, wq, wk, wv, wo):
    x2 = np.ascontiguousarray(np.asarray(x, dtype=np.float32)[0])  # [T, DIM]
    xT = np.ascontiguousarray(x2.T)                                # [DIM, T]
    cos = np.asarray(rope_cos, dtype=np.float32)                   # [T, 64]
    sin = np.asarray(rope_sin, dtype=np.float32)
    C = np.ascontiguousarray(np.repeat(cos.T, 2, axis=0))          # [128, T]
    S2 = np.repeat(sin.T, 2, axis=0)
    S2[0::2, :] *= -1.0
    S2 = np.ascontiguousarray(S2)
    msk = np.zeros((128, TPB, SC), dtype=np.float32)
    for o in range(TPB):
        r = np.arange(128)[:, None] + 128 * o
        j = np.arange(SC)[None, :]
        msk[:, o, :] = np.where(j >= r, 0.0, -30000.0)
    psw = np.zeros((128, 128), dtype=np.float32)
    k = np.arange(128)
    psw[k, k ^ 1] = 1.0
    ident = np.eye(128, dtype=np.float32)
    scale = 1.0 / np.sqrt(np.float32(HD))

    wq = np.asarray(wq, dtype=np.float32)
    wk = np.asarray(wk, dtype=np.float32) * scale
    wv = np.asarray(wv, dtype=np.float32)
    wo = np.asarray(wo, dtype=np.float32)
    in_maps = []
    for i in range(N_CORES):
        wq_i = wq[:, i * NQ * HD:(i + 1) * NQ * HD]
        wk_i = wk[:, i * HD:(i + 1) * HD]
        wv_i = wv[:, i * HD:(i + 1) * HD]
        wqkv_i = np.ascontiguousarray(np.concatenate([wq_i, wk_i, wv_i], axis=1))
        wo_i = np.ascontiguousarray(wo[i * NQ * HD:(i + 1) * NQ * HD, :])
        in_maps.append({
            "xT": xT, "wqkv": wqkv_i, "wo": wo_i,
            "ropeC": C, "ropeS2": S2, "masks": msk,
            "pswap": psw, "ident": ident,
        })
    return in_maps


_cached = {}


def _get_nc():
    if "nc" not in _cached:
        _install_fixups()
        _cached["nc"] = build()
    return _cached["nc"]


def kernel(x, rope_cos, rope_sin, wq, wk, wv, wo, _trace=False):
    nc = _get_nc()
    in_maps = host_prep(x, rope_cos, rope_sin, wq, wk, wv, wo)
    res = run_bass_kernel_spmd(nc, in_maps, core_ids=list(range(N_CORES)),
                               trace=_trace)
    acc = res.results[0]["outT"].astype(np.float32)
    for i in range(1, N_CORES):
        acc = acc + res.results[i]["outT"]
    out = np.ascontiguousarray(acc.T)[None]      # [1, T, DIM]
    if _trace:
        return out, res
    return out
